# revision 12
# baseline (speedup 1.0000x reference)
"""Trainium2 Bass kernel for nn_Concat_26147760898611.

Mean-pool over the word dim of article_concat [256, 2048, 300] and
options_concat [256, 64, 300], concat features -> [256, 600].

Sharding: pure data parallel over batch across 8 NeuronCores
(32 batches per core). Per core ~81 MB must stream from HBM; the 16
DMA engines sustain ~425 GB/s when nothing backpressures them, so the
whole kernel is built to keep the data-tile pool draining even while
HAM duty-cycle throttling halves engine clocks:

  - each article batch [2048, 300] is DMA'd as one 2.46 MB transfer into
    an SBUF tile [128 partitions, 16 words, 300 feat] (19.2 KB
    contiguous per partition -> striped across all 16 DMA engines;
    smaller per-partition transfers fall onto a single engine at
    ~26 GB/s, so no word-chunking anywhere), 8 tiles in flight.
  - ONE fold level halves the word axis and converts to bf16 in the
    same DVE/GpSimd pass (fp32 inputs cap the fold at 1x DVE speed, so
    deeper fp32 folds are a loss); batches alternate between DVE and
    GpSimd so either engine alone could almost keep up at half clock.
  - the 8 surviving word columns per batch are reduced across the
    partition dim by single-pass bf16 matmuls (4x fewer PE cycles than
    fp32's two half-rate passes) with a selector whose single nonzero
    column is 1/n_words (exact power of two in bf16), routing batch b's
    MEAN into PSUM row b. bf16 rounding of pair-sums keeps rel err
    ~2e-3, well under the 2e-2 gate.
  - results are stored DIRECTLY from PSUM to DRAM (no scalar drain on
    the critical path): options rows early, article rows 0..30 while
    the last batch streams, row 31 at the end.
  - the last batch gets a second (all-bf16, 2x-mode) fold so its
    post-last-byte tail is fold+fold+4 matmuls+one 1.2KB store.
  - the three leading data DMAs are dispatched before any selector
    setup so the stream starts as early as the queues allow; sel_a is
    built on-chip with memsets (no descriptor traffic at the head).
  - store DMAs dispatch from the Scalar engine queue so their
    semaphore waits never stall the Sync queue that feeds data DMAs.

Self-contained: hardcodes all shapes; no file reads.
"""

import numpy as np

N_CORES = 8
B = 256  # full batch
BC = B // N_CORES  # 32 batches per core
DIM = 300
AW = 2048  # article words per batch
OW = 64  # options words per batch
P = 128  # SBUF partitions
AWP = AW // P  # 16 article words per partition
OWP = 16  # options words per partition (4 batches x 16 words)
DATA_BUFS = 8
FOLD_BUFS = 3
WARMUP_MMS = 12

_CACHE = {}


def _build_nc():
    import concourse.bacc as bacc
    import concourse.mybir as mybir
    import concourse.tile as tile

    f32 = mybir.dt.float32
    bf16 = mybir.dt.bfloat16
    nc = bacc.Bacc("TRN2", target_bir_lowering=False, debug=False)

    art = nc.dram_tensor("article", [BC, AW, DIM], f32, kind="ExternalInput")
    opt = nc.dram_tensor("options", [BC, OW, DIM], f32, kind="ExternalInput")
    sel_o = nc.dram_tensor("sel_o", [P, BC], bf16, kind="ExternalInput")
    out = nc.dram_tensor("out", [BC, 2 * DIM], f32, kind="ExternalOutput")

    # [BC, 128, 16, 300]: partition p <- words p*16 .. p*16+15 (contiguous)
    art_r = art.ap().rearrange("b (p w) f -> b p w f", p=P)
    # [128, 16, 300]: partition p <- 16 consecutive words of batch p//4
    opt_r = opt.ap().rearrange("b (s q) f -> (b s) q f", s=P // BC)

    with tile.TileContext(nc) as tc:
        with (
            tc.tile_pool(name="const", bufs=1) as cpool,
            tc.tile_pool(name="data", bufs=DATA_BUFS) as dpool,
            tc.tile_pool(name="fold", bufs=FOLD_BUFS) as fpool,
            tc.tile_pool(name="outp", bufs=1) as opool,
            tc.tile_pool(name="psum", bufs=1, space="PSUM") as ppool,
        ):
            # data first: the Sync queue dispatches these before anything
            # else so the first bytes land as soon as the preamble ends
            t_opt = dpool.tile([P, OWP, DIM], f32, tag="data")
            nc.sync.dma_start(t_opt[:], opt_r[:])
            pre = []
            for b in range(2):
                t = dpool.tile([P, AWP, DIM], f32, tag="data")
                nc.sync.dma_start(t[:], art_r[b])
                pre.append(t)

            # sel_a built on-chip: a zero band whose single all-ones
            # column carries 1/AW (exact in bf16), so PSUM accumulates
            # the mean directly. sel_o (block-diagonal, 1/OW) comes in
            # via a small DMA since compute memsets must start on a
            # quadrant partition.
            sel_a_t = cpool.tile([P, 2 * BC - 1], bf16, tag="sel_a")
            nc.gpsimd.memset(sel_a_t[:], 0.0)
            nc.gpsimd.memset(sel_a_t[:, BC - 1 : BC], 1.0 / AW)
            sel_o_t = cpool.tile([P, BC], bf16, tag="sel_o")
            # dispatched from the Scalar queue: its 128 tiny descriptors
            # would otherwise eat issue slots of the big-descriptor stream
            nc.scalar.dma_start(sel_o_t[:], sel_o.ap()[:])

            psum_a1 = ppool.tile([BC, DIM], f32, tag="psum_a1")
            psum_a2 = ppool.tile([BC, DIM], f32, tag="psum_a2")
            psum_b = ppool.tile([BC, DIM], f32, tag="psum_b")
            psum_w = ppool.tile([BC, 2 * BC - 1], f32, tag="psum_w")

            # PE warmup: flip the HAM clock gate up before real data lands
            for _ in range(WARMUP_MMS):
                nc.tensor.matmul(
                    psum_w[:], sel_o_t[:], sel_a_t[:], start=True, stop=True
                )

            def fold_and_mm(t, nch, sel_ap, psum, first, last, eng, ftag):
                # halve the word axis once (f32 -> bf16), then one bf16
                # matmul per surviving word column
                n = nch // 2
                f = fpool.tile([P, n, DIM], bf16, tag=ftag)
                eng.tensor_add(f[:], t[:, 0:n, :], t[:, n : 2 * n, :])
                for j in range(n):
                    nc.tensor.matmul(
                        psum[:],
                        sel_ap,
                        f[:, j, :],
                        start=(first and j == 0),
                        stop=(last and j == n - 1),
                    )

            out_t = opool.tile([BC, 2 * DIM], f32, tag="out")
            out2_t = opool.tile([BC, DIM], f32, tag="out2")

            # options: fold + 8 matmuls -> psum_b holds the means
            # (PSUM can't source a DMA, so drains are plain copies)
            fold_and_mm(t_opt, OWP, sel_o_t[:], psum_b, True, True,
                        nc.vector, "fold8")
            nc.scalar.copy(out_t[:, DIM : 2 * DIM], psum_b[:])

            # article batches 0..30 accumulate means in psum_a1
            for b in range(BC - 1):
                if b < 2:
                    t = pre[b]
                else:
                    t = dpool.tile([P, AWP, DIM], f32, tag="data")
                    nc.sync.dma_start(t[:], art_r[b])
                eng = nc.vector if b % 2 == 0 else nc.gpsimd
                fold_and_mm(
                    t,
                    AWP,
                    sel_a_t[:, BC - 1 - b : 2 * BC - 1 - b],
                    psum_a1,
                    b == 0,
                    b == BC - 2,
                    eng,
                    "fold8",
                )

            # last batch into psum_a2: fold in two half-word pieces so the
            # first 4 matmuls pipeline with the second fold, shortening
            # the post-last-byte critical path
            t = dpool.tile([P, AWP, DIM], f32, tag="data")
            nc.sync.dma_start(t[:], art_r[BC - 1])
            sel_last = sel_a_t[:, 0:BC]
            for h in range(2):
                w0 = h * (AWP // 2)
                fh = fpool.tile([P, AWP // 4, DIM], bf16, tag=f"fold4_{h}")
                nc.vector.tensor_add(
                    fh[:],
                    t[:, w0 : w0 + AWP // 4, :],
                    t[:, w0 + AWP // 4 : w0 + AWP // 2, :],
                )
                for j in range(AWP // 4):
                    nc.tensor.matmul(
                        psum_a2[:],
                        sel_last,
                        fh[:, j, :],
                        start=(h == 0 and j == 0),
                        stop=(h == 1 and j == AWP // 4 - 1),
                    )

            # early drain: rows 0..30 (both halves) stored while the last
            # batch streams; all drains/stores live on the Scalar queue
            nc.scalar.copy(out_t[0 : BC - 1, 0:DIM], psum_a1[0 : BC - 1, :])
            nc.scalar.dma_start(out.ap()[0 : BC - 1, :], out_t[0 : BC - 1, :])

            # tail drain: copies must start at partition 0, so copy all of
            # psum_a2 (rows 0..30 are zeros) and store only row 31
            nc.scalar.copy(out2_t[:], psum_a2[:])
            nc.scalar.dma_start(out.ap()[BC - 1 : BC, 0:DIM],
                                out2_t[BC - 1 : BC, :])
            nc.scalar.dma_start(out.ap()[BC - 1 : BC, DIM : 2 * DIM],
                                out_t[BC - 1 : BC, DIM : 2 * DIM])

    nc.compile()
    return nc


def get_nc():
    if "nc" not in _CACHE:
        _CACHE["nc"] = _build_nc()
    return _CACHE["nc"]


def make_in_maps(article, options):
    import ml_dtypes

    article = np.ascontiguousarray(np.asarray(article, dtype=np.float32))
    options = np.ascontiguousarray(np.asarray(options, dtype=np.float32))
    assert article.shape == (B, AW, DIM), article.shape
    assert options.shape == (B, OW, DIM), options.shape
    sel_o = np.zeros((P, BC), np.float32)
    sel_o[np.arange(P), np.arange(P) // (P // BC)] = 1.0 / OW
    sel_o = sel_o.astype(ml_dtypes.bfloat16)
    return [
        {
            "article": article[i * BC : (i + 1) * BC],
            "options": options[i * BC : (i + 1) * BC],
            "sel_o": sel_o,
        }
        for i in range(N_CORES)
    ]


def run_sharded(article, options, **spmd_kwargs):
    from concourse.bass_utils import run_bass_kernel_spmd

    nc = get_nc()
    in_maps = make_in_maps(article, options)
    res = run_bass_kernel_spmd(nc, in_maps, list(range(N_CORES)), **spmd_kwargs)
    full = np.concatenate(
        [res.results[i]["out"] for i in range(N_CORES)], axis=0
    ).astype(np.float32)
    return full, res


def kernel(article_concat, options_concat):
    full, _ = run_sharded(article_concat, options_concat)
    return full
